# revision 2
# baseline (speedup 1.0000x reference)
"""3-layer GraphSAGE (mean agg, sum combine) on 8 Trainium2 NeuronCores.

Sharding: core m owns dst rows [m*B_l, (m+1)*B_l) of each layer's output.
Edges partitioned by dst, sorted (dst-block, src-chunk); per-(block,chunk)
runs padded to a core-uniform schedule so all 8 cores share one instruction
stream (SPMD). Node tables replicated bf16 [n,128] (256B rows), gathered per
edge via SWDGE dma_gather (int16 idx within <=32k-row chunks, <=1024
idx/call on 4 queues). Segment sum = one-hot matmuls accumulating
aggT[hid,seg] in PSUM per 128-dst block. Combine per block:
  out = relu?( (aggT/cnt).T @ Wneigh + h_dst @ Wself + b )
with the neigh psum scaled by 1/cnt via ACT per-partition scale, and
h_dst rows read with a partition_id-dependent dynamic DMA. Shards are
AllGather'd between layers. Layer0 gathers padded feature rows
[x(16),1,0...] (256B); DMA-transpose yields xT; K=17 matmul vs
[W_init;b_init] then relu gives per-edge messages.
"""

import sys

sys.path.insert(0, "/opt/trn_rl_repo")

import numpy as np
import ml_dtypes
from contextlib import ExitStack

import concourse.bacc as bacc
import concourse.bass as bass
import concourse.mybir as mybir
from concourse.tile import TileContext

NCORES = 8
BF16 = mybir.dt.bfloat16
F32 = mybir.dt.float32
I16 = mybir.dt.int16
U8 = mybir.dt.uint8

CALL_IDX = 1024
CHUNK = 32768
NQ = 4

N0, N1, N2, N3 = 200000, 100000, 50000, 25000
IN_DIM, HID = 16, 128

_CACHE = {}


def _pad128(x):
    return (np.asarray(x) + 127) // 128 * 128


class LayerPlan:
    def __init__(self, src_all, dst_all, n_in_rows, n_out, relu):
        self.relu = relu
        B = n_out // NCORES
        self.B = B
        self.nblocks = (B + 127) // 128
        self.Bpad = self.nblocks * 128
        self.nchunks = (n_in_rows + CHUNK - 1) // CHUNK
        self.n_in_rows = n_in_rows
        src = np.asarray(src_all, np.int64)
        dst = np.asarray(dst_all, np.int64)

        per_core = []
        for m in range(NCORES):
            sel = (dst >= m * B) & (dst < (m + 1) * B)
            s, d = src[sel], dst[sel] - m * B
            blk = d >> 7
            chk = s // CHUNK
            order = np.lexsort((s, chk, blk))
            per_core.append((s[order], d[order], blk[order], chk[order]))

        cnt = np.zeros((NCORES, self.nblocks, self.nchunks), np.int64)
        for m in range(NCORES):
            s, d, blk, chk = per_core[m]
            np.add.at(cnt[m], (blk, chk), 1)
        runs = _pad128(cnt.max(axis=0))
        zero = runs.sum(1) == 0
        runs[zero, 0] = 128
        self.runs = runs
        self.total = int(runs.sum())

        self.idx = np.zeros((NCORES, self.total), np.int64)
        self.dsub = np.full((NCORES, self.total), -1.0, np.float32)
        starts = np.zeros((self.nblocks, self.nchunks), np.int64)
        pos = 0
        for b in range(self.nblocks):
            for c in range(self.nchunks):
                starts[b, c] = pos
                pos += int(runs[b, c])
        for m in range(NCORES):
            s, d, blk, chk = per_core[m]
            # positions within each (blk, chk) run
            key = blk * self.nchunks + chk
            within = np.zeros(len(s), np.int64)
            if len(s):
                brk = np.flatnonzero(np.diff(key)) + 1
                seg_starts = np.concatenate(([0], brk))
                seg_ids = np.repeat(np.arange(len(seg_starts)),
                                    np.diff(np.concatenate((seg_starts,
                                                            [len(s)]))))
                within = np.arange(len(s)) - seg_starts[seg_ids]
            p = starts[blk, chk] + within
            self.idx[m, :] = 0
            # default pads: chunk base row per schedule slot
            for b in range(self.nblocks):
                for c in range(self.nchunks):
                    s0 = starts[b, c]
                    self.idx[m, s0 : s0 + int(runs[b, c])] = c * CHUNK
            self.idx[m, p] = s
            self.dsub[m, :] = -1.0
            self.dsub[m, p] = d & 127

        self.inv = np.zeros((NCORES, self.Bpad), np.float32)
        for m in range(NCORES):
            _, d, _, _ = per_core[m]
            c = np.bincount(d, minlength=self.Bpad).astype(np.float32)
            self.inv[m] = 1.0 / np.maximum(c, 1.0)

        # gather calls: contiguous schedule spans within one chunk, <=CALL_IDX
        self.calls = []
        for b in range(self.nblocks):
            for c in range(self.nchunks):
                r = int(runs[b, c])
                s0 = int(starts[b, c])
                while r > 0:
                    take = min(r, CALL_IDX)
                    if (self.calls and self.calls[-1][2] == c
                            and self.calls[-1][0] + self.calls[-1][1] == s0
                            and self.calls[-1][1] + take <= CALL_IDX):
                        self.calls[-1] = (self.calls[-1][0],
                                          self.calls[-1][1] + take, c)
                    else:
                        self.calls.append((s0, take, c))
                    s0 += take
                    r -= take

    def wrapped_idx(self):
        out = np.zeros((NCORES, 128, self.total // 16), np.int16)
        for m in range(NCORES):
            for s0, n0, c0 in self.calls:
                seg = (self.idx[m, s0 : s0 + n0] - c0 * CHUNK).astype(np.int16)
                a = seg.reshape(n0 // 16, 16).T
                out[m, :, s0 // 16 : (s0 + n0) // 16] = np.tile(a, (8, 1))
        return out

    def dsub_bf16(self):
        out = self.dsub.reshape(NCORES, self.total // 128, 128).transpose(0, 2, 1)
        return np.ascontiguousarray(out).astype(ml_dtypes.bfloat16)

    def inv_cols(self):
        # [NCORES, 128, nblocks]: inv[dst=b*128+p] at [:, p, b]
        return np.ascontiguousarray(
            self.inv.reshape(NCORES, self.nblocks, 128).transpose(0, 2, 1))


def build(p0, p1, p2):
    nc = bacc.Bacc(num_devices=NCORES, num_swdge_queues=NQ)

    ftab = nc.declare_dram_parameter("ftab", [N0, 128], BF16, isOutput=False)
    NB = 256 * 3 + 256 + 256 + 256
    cblob = nc.declare_dram_parameter("cblob", [128, NB], U8, isOutput=False)

    plans = [p0, p1, p2]
    eparams = []
    for li, p in enumerate(plans):
        iw = nc.declare_dram_parameter(f"idx{li}", [128, p.total // 16], I16,
                                       isOutput=False)
        dw = nc.declare_dram_parameter(f"dsub{li}", [128, p.total // 128], BF16,
                                       isOutput=False)
        inv = nc.declare_dram_parameter(f"inv{li}", [128, p.nblocks], F32,
                                        isOutput=False)
        eparams.append((iw, dw, inv))

    out = nc.declare_dram_parameter("out", [p2.Bpad, 128], F32, isOutput=True)

    h1_sh = nc.dram_tensor("h1_sh", [p0.Bpad, 128], BF16)
    h1_full = nc.dram_tensor("h1_full", [p0.Bpad * NCORES, 128], BF16,
                             addr_space="Shared")
    h2_sh = nc.dram_tensor("h2_sh", [p1.Bpad, 128], BF16)
    h2_full = nc.dram_tensor("h2_full", [p1.Bpad * NCORES, 128], BF16,
                             addr_space="Shared")
    RG = [list(range(NCORES))]

    with TileContext(nc) as tc:
        with ExitStack() as ctx:
            consts = ctx.enter_context(tc.tile_pool(name="consts", bufs=1))
            idxp = ctx.enter_context(tc.tile_pool(name="idxp", bufs=1))
            gp = ctx.enter_context(tc.tile_pool(name="gp", bufs=12))
            xp = ctx.enter_context(tc.tile_pool(name="xp", bufs=4))
            mp = ctx.enter_context(tc.tile_pool(name="mp", bufs=4))
            ohp = ctx.enter_context(tc.tile_pool(name="ohp", bufs=4))
            nodp = ctx.enter_context(tc.tile_pool(name="nodp", bufs=3))
            psA = ctx.enter_context(tc.tile_pool(name="psA", bufs=2,
                                                 space="PSUM"))
            psF = ctx.enter_context(tc.tile_pool(name="psF", bufs=2,
                                                 space="PSUM"))
            psN = ctx.enter_context(tc.tile_pool(name="psN", bufs=2,
                                                 space="PSUM"))

            cb = consts.tile([128, NB], U8)
            nc.sync.dma_start(out=cb[:], in_=cblob[:])
            w17_t = cb[:, 0:256].bitcast(BF16)        # [W_init;b_init] rows 0:17
            wself_t = cb[:, 256:512].bitcast(BF16)
            wneigh_t = cb[:, 512:768].bitcast(BF16)
            iota_t = cb[:, 768:1024].bitcast(BF16)    # [128,128] iota rows
            brow_t = cb[0:1, 1024:1280].bitcast(BF16)  # b_self+b_neigh
            ones_t = cb[0:1, 1280:1536].bitcast(BF16)

            pid = nc.sync.partition_id()

            def layer(li, p, table, self_tab, self_base, out_sh, out_dtype):
                iw, dw, invw = eparams[li]
                idx_t = idxp.tile([128, p.total // 16], I16, tag=f"idx{li}")
                nc.sync.dma_start(out=idx_t[:], in_=iw[:])
                dsub_t = idxp.tile([128, p.total // 128], BF16, tag=f"ds{li}")
                nc.sync.dma_start(out=dsub_t[:], in_=dw[:])
                inv_t = idxp.tile([128, p.nblocks], F32, tag=f"inv{li}")
                nc.sync.dma_start(out=inv_t[:], in_=invw[:])

                # all gather calls up-front; Tile throttles via pool slots
                tile_src = [None] * (p.total // 128)
                for ci, (s0, n0, c0) in enumerate(p.calls):
                    g = gp.tile([128, CALL_IDX // 128, 128], BF16,
                                tag="g")
                    hi = min((c0 + 1) * CHUNK, p.n_in_rows)
                    nc.gpsimd.dma_gather(
                        out_ap=g[:, : n0 // 128, :],
                        in_ap=table[c0 * CHUNK : hi, :],
                        idxs_ap=idx_t[:, s0 // 16 : (s0 + n0) // 16],
                        num_idxs=n0,
                        num_idxs_reg=n0,
                        elem_size=128,
                        queue_num=ci % NQ,
                    )
                    for k in range(n0 // 128):
                        tile_src[s0 // 128 + k] = (g, k)

                tpos = 0
                for b in range(p.nblocks):
                    ntb = int(p.runs[b].sum()) // 128
                    agg_ps = psA.tile([128, 128], F32, tag="agg")
                    for tb in range(ntb):
                        g, slot = tile_src[tpos]
                        tcol = tpos
                        tpos += 1
                        if li == 0:
                            xT = xp.tile([128, 128], BF16, tag="xT")
                            nc.sync.dma_start_transpose(out=xT[:],
                                                        in_=g[:, slot, :])
                            fps = psF.tile([128, 128], F32, tag="fc")
                            nc.tensor.matmul(fps[:], xT[0:17, :],
                                             w17_t[0:17, :],
                                             start=True, stop=True)
                            msgs = mp.tile([128, 128], BF16, tag="msgs")
                            nc.scalar.activation(
                                out=msgs[:], in_=fps[:],
                                func=mybir.ActivationFunctionType.Relu)
                            lhs_ap = msgs[:]
                        else:
                            lhs_ap = g[:, slot, :]
                        oh = ohp.tile([128, 128], BF16, tag="oh")
                        nc.vector.tensor_tensor(
                            out=oh[:], in0=iota_t[:, :],
                            in1=dsub_t[:, tcol : tcol + 1].to_broadcast(
                                [128, 128]),
                            op=mybir.AluOpType.is_equal)
                        nc.tensor.matmul(agg_ps[:], lhs_ap, oh[:],
                                         start=(tb == 0), stop=(tb == ntb - 1))

                    # ---- block combine ----
                    aggT_sb = nodp.tile([128, 128], BF16, tag="at")
                    nc.vector.tensor_copy(out=aggT_sb[:], in_=agg_ps[:])
                    # neigh raw psum [seg, hid]
                    nps = psN.tile([128, 128], F32, tag="nps")
                    nc.tensor.matmul(nps[:], aggT_sb[:], wneigh_t[:],
                                     start=True, stop=True)
                    # self+bias psum [seg, hid]
                    sps = psN.tile([128, 128], F32, tag="sps")
                    nc.tensor.matmul(sps[:], ones_t[:, :], brow_t[:, :],
                                     start=True, stop=False)
                    hd = nodp.tile([128, 128], BF16, tag="hd")
                    if li == 0:
                        nc.sync.dma_start(
                            out=hd[:],
                            in_=self_tab[bass.ds(self_base + b * 128, 128), :])
                        xd = nodp.tile([128, 128], BF16, tag="xd")
                        nc.sync.dma_start_transpose(out=xd[:], in_=hd[:])
                        fpd = psF.tile([128, 128], F32, tag="fc")
                        nc.tensor.matmul(fpd[:], w17_t[0:17, :], xd[0:17, :],
                                         start=True, stop=True)
                        hdT = nodp.tile([128, 128], BF16, tag="hdT")
                        nc.scalar.activation(
                            out=hdT[:], in_=fpd[:],
                            func=mybir.ActivationFunctionType.Relu)
                    else:
                        nc.sync.dma_start(
                            out=hd[:],
                            in_=self_tab[bass.ds(self_base + b * 128, 128), :])
                        hdT = nodp.tile([128, 128], BF16, tag="hdT")
                        nc.sync.dma_start_transpose(out=hdT[:], in_=hd[:])
                    nc.tensor.matmul(sps[:], hdT[:], wself_t[:],
                                     start=False, stop=True)
                    # neigh * inv (ACT per-partition scale) -> SBUF f32
                    nsb = nodp.tile([128, 128], F32, tag="nsb")
                    nc.scalar.activation(
                        out=nsb[:], in_=nps[:],
                        func=mybir.ActivationFunctionType.Copy,
                        scale=inv_t[:, b : b + 1])
                    ob = nodp.tile([128, 128], out_dtype, tag=f"ob{li}")
                    if p.relu:
                        tmp = nodp.tile([128, 128], F32, tag="tmp")
                        nc.vector.tensor_tensor(out=tmp[:], in0=sps[:],
                                                in1=nsb[:],
                                                op=mybir.AluOpType.add)
                        nc.scalar.activation(
                            out=ob[:], in_=tmp[:],
                            func=mybir.ActivationFunctionType.Relu)
                    else:
                        nc.vector.tensor_tensor(out=ob[:], in0=sps[:],
                                                in1=nsb[:],
                                                op=mybir.AluOpType.add)
                    nc.sync.dma_start(out=out_sh[b * 128 : (b + 1) * 128, :],
                                      in_=ob[:])

            base0 = pid * p0.B
            base1 = (pid // 2) * p0.Bpad + (pid % 2) * p1.B
            base2 = (pid // 2) * p1.Bpad + (pid % 2) * p2.B

            layer(0, p0, ftab, ftab, base0, h1_sh, BF16)
            nc.gpsimd.collective_compute(
                "AllGather", mybir.AluOpType.bypass, replica_groups=RG,
                ins=[h1_sh[:]], outs=[h1_full[:]])
            layer(1, p1, h1_full, h1_full, base1, h2_sh, BF16)
            nc.gpsimd.collective_compute(
                "AllGather", mybir.AluOpType.bypass, replica_groups=RG,
                ins=[h2_sh[:]], outs=[h2_full[:]])
            layer(2, p2, h2_full, h2_full, base2, out, F32)

    nc.compile()
    return nc


def _prep(features, W_init, b_init, W_self, b_self, W_neigh, b_neigh,
          src0, dst0, src1, dst1, src2, dst2):
    p0 = LayerPlan(src0, dst0, N0, N1, relu=True)
    p1_src = np.asarray(src1, np.int64)
    remap1 = (p1_src // p0.B) * p0.Bpad + p1_src % p0.B
    p1 = LayerPlan(remap1, dst1, p0.Bpad * NCORES, N2, relu=True)
    p2_src = np.asarray(src2, np.int64)
    remap2 = (p2_src // p1.B) * p1.Bpad + p2_src % p1.B
    p2 = LayerPlan(remap2, dst2, p1.Bpad * NCORES, N3, relu=False)

    bf = ml_dtypes.bfloat16
    ftab = np.zeros((N0, 128), bf)
    ftab[:, :IN_DIM] = features.astype(bf)
    ftab[:, IN_DIM] = np.ones((), bf)

    w17 = np.zeros((128, 128), np.float32)
    w17[:IN_DIM, :] = W_init
    w17[IN_DIM, :] = b_init
    NB = 256 * 3 + 256 + 256 + 256
    cblob = np.zeros((128, NB), np.uint8)
    cblob[:, 0:256] = w17.astype(bf).view(np.uint8)
    cblob[:, 256:512] = W_self.astype(bf).view(np.uint8)
    cblob[:, 512:768] = W_neigh.astype(bf).view(np.uint8)
    iota = np.tile(np.arange(128, dtype=np.float32), (128, 1)).astype(bf)
    cblob[:, 768:1024] = iota.view(np.uint8)
    brow = (np.asarray(b_self) + np.asarray(b_neigh)).astype(bf).reshape(1, 128)
    cblob[0:1, 1024:1280] = brow.view(np.uint8)
    cblob[0:1, 1280:1536] = np.ones((1, 128), bf).view(np.uint8)

    in_common = dict(ftab=ftab, cblob=cblob)
    per_core = []
    for li, p in enumerate((p0, p1, p2)):
        iw = p.wrapped_idx()
        dw = p.dsub_bf16()
        iv = p.inv_cols()
        per_core.append((f"idx{li}", iw, f"dsub{li}", dw, f"inv{li}", iv))
    in_maps = []
    for m in range(NCORES):
        d = dict(in_common)
        for (ni, iw, nd, dw, nv, iv) in per_core:
            d[ni] = iw[m]
            d[nd] = dw[m]
            d[nv] = iv[m].astype(np.float32)
        in_maps.append(d)
    return p0, p1, p2, in_maps


def kernel(**inputs):
    features = np.asarray(inputs["features"], np.float32)
    args = (features, np.asarray(inputs["W_init"], np.float32),
            np.asarray(inputs["b_init"], np.float32),
            np.asarray(inputs["W_self"], np.float32),
            np.asarray(inputs["b_self"], np.float32),
            np.asarray(inputs["W_neigh"], np.float32),
            np.asarray(inputs["b_neigh"], np.float32),
            np.asarray(inputs["src0"]), np.asarray(inputs["dst0"]),
            np.asarray(inputs["src1"]), np.asarray(inputs["dst1"]),
            np.asarray(inputs["src2"]), np.asarray(inputs["dst2"]))
    p0, p1, p2, in_maps = _prep(*args)

    if "nc" not in _CACHE:
        _CACHE["nc"] = build(p0, p1, p2)
    nc = _CACHE["nc"]
    _CACHE["in_maps"] = in_maps

    from concourse.bass_utils import run_bass_kernel_spmd

    res = run_bass_kernel_spmd(nc, in_maps, list(range(NCORES)),
                               trace=bool(_CACHE.get("trace")))
    _CACHE["last_result"] = res
    outp = np.concatenate(
        [res.results[m]["out"][: N3 // NCORES] for m in range(NCORES)], axis=0)
    return outp.astype(np.float32)

